# revision 13
# baseline (speedup 1.0000x reference)
"""Trainium2 Bass kernel for nn_EquivariantScalarMLP (data-parallel over 8 cores).

Per-core layout (8192 samples, 64 tiles x 128 samples):
  - FiLM/MLP chain runs feature-on-partition (activations transposed via PE).
    FiLM biases ride the weight matrices (ones-row augmented cond transpose),
    so Act-engine PSUM->SBUF copies need no bias and run one instr per FiLM.
  - Hypernet weights w = s3 @ wemb land batch-on-partition in PSUM; wemb's
    columns are host-permuted to (l, o, i) order so the DVE mix reads w
    contiguously. Act copies w to SBUF as fp16.
  - Equivariant mix y[b,o,d] = sum_i w[b,(l,o,i)] * x[b,(l,d,i)] runs on DVE
    in fp16 with packed-innermost access patterns (2x perf mode): one
    broadcast multiply per l-block building t[b,(d,o,i)], then out-of-place
    ping-pong fold-adds over i; the last fold writes f32 directly into the
    output layout.
  - All input transposes + inv readout share one PSUM bank per tile.
  - e3nn 1/sqrt(32) norm folded into wemb on host.
"""

import numpy as np
import ml_dtypes

B = 65536
NCORES = 8
BPC = B // NCORES          # 8192 samples per core
NT = BPC // 128            # 64 tiles per core
S_IN = 256
S_OUT = 128
MUL = 32
COND = 64
D1, D2 = 3, 5

_cache = {}


def _build():
    from contextlib import ExitStack
    import concourse.bass as bass
    from concourse import bacc
    import concourse.tile as tile
    import concourse.mybir as mybir

    f32 = mybir.dt.float32
    f16 = mybir.dt.float16
    AF = mybir.ActivationFunctionType

    nc = bacc.Bacc()

    x_d = nc.dram_tensor("x", [BPC, 512], f32, kind="ExternalInput")
    c_d = nc.dram_tensor("cond", [BPC, COND], f32, kind="ExternalInput")
    f1w_d = nc.dram_tensor("f1w", [COND + 1, 512], f16, kind="ExternalInput")
    f2w_d = nc.dram_tensor("f2w", [COND + 1, 512], f16, kind="ExternalInput")
    fcw_d = nc.dram_tensor("fcw", [S_IN, S_IN], f16, kind="ExternalInput")
    invw_d = nc.dram_tensor("invw", [S_IN, S_OUT], f16, kind="ExternalInput")
    wembw_d = nc.dram_tensor("wembw", [S_IN, 2048], f16, kind="ExternalInput")
    ident_d = nc.dram_tensor("ident", [128, 128], f32, kind="ExternalInput")
    out_d = nc.dram_tensor("out", [BPC, 384], f32, kind="ExternalOutput")

    with tile.TileContext(nc) as tc, ExitStack() as ctx:
        wpool = ctx.enter_context(tc.tile_pool(name="weights", bufs=1))
        # --- persistent weights in SBUF ---
        f1w = wpool.tile([COND + 1, 512], f16)
        nc.sync.dma_start(f1w[:], f1w_d[:])
        f2w = wpool.tile([COND + 1, 512], f16)
        nc.sync.dma_start(f2w[:], f2w_d[:])
        # fc1_w [256,256] -> [128, (kchunk 2, m 256)]
        fcw = wpool.tile([128, 512], f16)
        nc.sync.dma_start(fcw[:].rearrange("k (c m) -> k c m", c=2),
                          fcw_d.rearrange("(c k) m -> k c m", c=2))
        invw = wpool.tile([128, 256], f16)
        nc.sync.dma_start(invw[:].rearrange("k (c m) -> k c m", c=2),
                          invw_d.rearrange("(c k) m -> k c m", c=2))
        wemb = wpool.tile([128, 4096], f16)
        nc.sync.dma_start(wemb[:].rearrange("k (c m) -> k c m", c=2),
                          wembw_d.rearrange("(c k) m -> k c m", c=2))
        ident = wpool.tile([128, 128], f32)
        nc.sync.dma_start(ident[:], ident_d[:])

        inp = ctx.enter_context(tc.tile_pool(name="inp", bufs=3))
        act = ctx.enter_context(tc.tile_pool(name="act", bufs=3))
        wsb = ctx.enter_context(tc.tile_pool(name="wsb", bufs=2))
        tpl = ctx.enter_context(tc.tile_pool(name="tpl", bufs=2))
        outp = ctx.enter_context(tc.tile_pool(name="outp", bufs=3))
        ps_in = ctx.enter_context(tc.tile_pool(name="ps_in", bufs=2, space="PSUM"))
        ps_g = ctx.enter_context(tc.tile_pool(name="ps_g", bufs=2, space="PSUM"))
        ps_w = ctx.enter_context(tc.tile_pool(name="ps_w", bufs=1, space="PSUM"))

        for t in range(NT):
            n0 = t * 128
            feat = inp.tile([128, 512], f32, tag="feat")
            nc.sync.dma_start(feat[:], x_d[n0:n0 + 128, :])
            cnd = inp.tile([128, COND], f32, tag="cnd")
            nc.sync.dma_start(cnd[:], c_d[n0:n0 + 128, :])

            # equiv part -> xd[b, (l, d, i)] fp16 (batch-on-partition), Act engine
            xd = act.tile([128, 256], f16, tag="xd")
            for l, D, src0, dst0 in ((0, D1, 256, 0), (1, D2, 256 + MUL * D1, MUL * D1)):
                nc.scalar.activation(
                    xd[:, dst0:dst0 + MUL * D].rearrange("p (d i) -> p d i", i=MUL),
                    feat[:, src0:src0 + MUL * D].rearrange("p (i d) -> p d i", d=D),
                    AF.Copy)

            # input transposes share one PSUM bank, released early.
            T = ps_in.tile([128, 384], f32, tag="T")
            for cblk in range(2):
                nc.tensor.transpose(T[:, 128 * cblk:128 * (cblk + 1)],
                                    feat[:, 128 * cblk:128 * (cblk + 1)], ident[:])
            s0T = act.tile([128, 256], f16, tag="s0T")
            nc.scalar.activation(s0T[:], T[:, 0:256], AF.Copy)
            nc.tensor.transpose(T[:COND, 256:384], cnd[:], ident[:])
            cT = act.tile([COND + 1, 128], f16, tag="cT")
            nc.scalar.activation(cT[:COND, :], T[:COND, 256:384], AF.Copy)
            nc.gpsimd.memset(cT[COND:COND + 1, :], 1.0)

            # --- FiLM1: gb1 = [film1_w; film1_b].T @ [cT; ones] ---
            gb1 = ps_g.tile([128, 512], f32, tag="gb")
            for m in range(4):
                nc.tensor.matmul(gb1[:, 128 * m:128 * (m + 1)],
                                 f1w[:, 128 * m:128 * (m + 1)], cT[:],
                                 start=True, stop=True)
            gbs1 = act.tile([128, 512], f16, tag="gbs1")
            nc.scalar.activation(gbs1[:], gb1[:], AF.Copy)
            s1T = act.tile([128, 256], f16, tag="s1T")
            nc.gpsimd.tensor_mul(s1T[:], s0T[:], gbs1[:, 0:256])
            nc.gpsimd.tensor_add(s1T[:], s1T[:], gbs1[:, 256:512])

            # --- fc1: s2 = fc1_w.T @ s1T ---
            s2 = ps_g.tile([128, 512], f32, tag="gb")
            for m in range(2):
                for c in range(2):
                    nc.tensor.matmul(s2[:, 128 * m:128 * (m + 1)],
                                     fcw[:, 256 * c + 128 * m:256 * c + 128 * (m + 1)],
                                     s1T[:, 128 * c:128 * (c + 1)],
                                     start=(c == 0), stop=(c == 1))
            s2sb = act.tile([128, 256], f16, tag="s2sb")
            nc.scalar.activation(s2sb[:], s2[:, 0:256], AF.Copy)

            # --- FiLM2 ---
            gb2 = ps_g.tile([128, 512], f32, tag="gb")
            for m in range(4):
                nc.tensor.matmul(gb2[:, 128 * m:128 * (m + 1)],
                                 f2w[:, 128 * m:128 * (m + 1)], cT[:],
                                 start=True, stop=True)
            gbs2 = act.tile([128, 512], f16, tag="gbs2")
            nc.scalar.activation(gbs2[:], gb2[:], AF.Copy)
            s3T = act.tile([128, 256], f16, tag="s3T")
            nc.gpsimd.tensor_mul(s3T[:], s2sb[:], gbs2[:, 0:256])
            nc.gpsimd.tensor_add(s3T[:], s3T[:], gbs2[:, 256:512])

            # --- invariant readout + transpose back (PSUM via gb rotation) ---
            To = ps_g.tile([128, 512], f32, tag="gb")
            for c in range(2):
                nc.tensor.matmul(To[:, 0:128], invw[:, 128 * c:128 * (c + 1)],
                                 s3T[:, 128 * c:128 * (c + 1)],
                                 start=(c == 0), stop=(c == 1))
            oT = act.tile([128, 128], f32, tag="oT")
            nc.scalar.activation(oT[:], To[:, 0:128], AF.Copy)
            ostage = outp.tile([128, 384], f32, tag="ostage")
            nc.tensor.transpose(To[:, 128:256], oT[:], ident[:])
            nc.scalar.activation(ostage[:, 0:128], To[:, 128:256], AF.Copy)

            # --- hypernet weights: w = s3 @ wemb, batch-on-partition ---
            w_sb = wsb.tile([128, 2048], f16, tag="w_sb")
            for half in range(2):
                wps = ps_w.tile([128, 1024], f32, tag=f"wps{half}")
                for c in range(2):
                    for s in range(2):
                        nc.tensor.matmul(wps[:, 512 * s:512 * (s + 1)],
                                         s3T[:, 128 * c:128 * (c + 1)],
                                         wemb[:, 2048 * c + 1024 * half + 512 * s:
                                              2048 * c + 1024 * half + 512 * (s + 1)],
                                         start=(c == 0), stop=(c == 1))
                nc.scalar.activation(w_sb[:, 1024 * half:1024 * (half + 1)], wps[:], AF.Copy)

            # --- per-sample equivariant mix on DVE (fp16, 2x perf mode) ---
            # ta[b, d, o, i] = w[b, (l,o,i)] * xd[b, (l,d,i)], then ping-pong
            # out-of-place fold-adds over i. The two l-block chains are
            # interleaved so consecutive DVE instrs are independent; the two
            # smallest folds go to GpSimd.
            st = {}
            for l, D, woff, xoff, yoff in ((0, D1, 0, 0, 128), (1, D2, 1024, MUL * D1, 128 + MUL * D1)):
                ta = tpl.tile([128, D * 1024], f16, tag=f"ta{l}")
                tb = tpl.tile([128, D * 512], f16, tag=f"tb{l}")
                st[l] = (ta, tb, D, woff, xoff, yoff)

            def mul(l):
                ta, tb, D, woff, xoff, yoff = st[l]
                tav = ta[:].rearrange("p (d o i) -> p d o i", o=MUL, i=MUL)
                wv = w_sb[:, woff:woff + 1024].rearrange("p (o i) -> p o i", i=MUL) \
                    .unsqueeze(1).broadcast_to((128, D, MUL, MUL))
                xv = xd[:, xoff:xoff + MUL * D].rearrange("p (d i) -> p d i", i=MUL) \
                    .unsqueeze(2).broadcast_to((128, D, MUL, MUL))
                nc.vector.tensor_mul(tav, wv, xv)

            def fold(l, level):
                ta, tb, D, woff, xoff, yoff = st[l]
                h = MUL >> level          # output i-extent of this fold
                src, dst = (ta, tb) if level % 2 == 1 else (tb, ta)
                sv = src[:, 0:D * 64 * h].rearrange("p (d o i) -> p d o i", o=MUL, i=2 * h)
                if h >= 2:
                    dv = dst[:, 0:D * 32 * h].rearrange("p (d o i) -> p d o i", o=MUL, i=h)
                    eng = nc.vector if h > 2 else nc.gpsimd
                    eng.tensor_add(dv, sv[:, :, :, 0:h], sv[:, :, :, h:2 * h])
                else:
                    yv = ostage[:, yoff:yoff + MUL * D] \
                        .rearrange("p (o d) -> p d o", d=D).unsqueeze(3)
                    nc.gpsimd.tensor_add(yv, sv[:, :, :, 0:1], sv[:, :, :, 1:2])

            mul(0)
            mul(1)
            for level in range(1, 6):
                fold(0, level)
                fold(1, level)

            nc.sync.dma_start(out_d[n0:n0 + 128, :], ostage[:])

    return nc


def kernel(features, conditioning_tensor, film1_w, film1_b, fc1_w,
           film2_w, film2_b, inv_w, wemb_w):
    from concourse.bass_utils import run_bass_kernel_spmd

    if "nc" not in _cache:
        nc = _build()
        if not nc.is_finalized():
            nc.finalize()
        _cache["nc"] = nc
    nc = _cache["nc"]

    norm = np.float32(1.0 / np.sqrt(MUL))
    feats = np.ascontiguousarray(np.asarray(features, np.float32))
    conds = np.ascontiguousarray(np.asarray(conditioning_tensor, np.float32))

    # wemb columns permuted to (l, o, i) layout; norm folded in.
    wemb_np = np.asarray(wemb_w, np.float32) * norm
    perm = np.empty(2048, dtype=np.int64)
    for l in range(2):
        base = l * MUL * MUL
        for o in range(MUL):
            for i in range(MUL):
                perm[base + o * MUL + i] = base + i * MUL + o
    wemb_perm = wemb_np[:, perm]

    f1wa = np.concatenate([np.asarray(film1_w, np.float32),
                           np.asarray(film1_b, np.float32)[None, :]], axis=0)
    f2wa = np.concatenate([np.asarray(film2_w, np.float32),
                           np.asarray(film2_b, np.float32)[None, :]], axis=0)

    shared = {
        "f1w": f1wa.astype(np.float16),
        "f2w": f2wa.astype(np.float16),
        "fcw": np.asarray(fc1_w, np.float32).astype(np.float16),
        "invw": np.asarray(inv_w, np.float32).astype(np.float16),
        "wembw": wemb_perm.astype(np.float16),
        "ident": np.eye(128, dtype=np.float32),
    }
    in_maps = []
    for i in range(NCORES):
        m = dict(shared)
        m["x"] = feats[i * BPC:(i + 1) * BPC]
        m["cond"] = conds[i * BPC:(i + 1) * BPC]
        in_maps.append(m)

    import os
    trace = bool(int(os.environ.get("KERNEL_TRACE", "0")))
    res = run_bass_kernel_spmd(nc, in_maps, core_ids=list(range(NCORES)), trace=trace)
    _cache["last"] = res
    return np.concatenate([r["out"] for r in res.results], axis=0)


# revision 14
# speedup vs baseline: 1.0489x; 1.0489x over previous
"""Trainium2 Bass kernel for nn_EquivariantScalarMLP (data-parallel over 8 cores).

Per-core layout (8192 samples, 64 tiles x 128 samples):
  - FiLM/MLP chain runs feature-on-partition (activations transposed via PE).
    FiLM biases ride the weight matrices (ones-row augmented cond transpose),
    so Act-engine PSUM->SBUF copies need no bias and run one instr per FiLM.
  - Hypernet weights w = s3 @ wemb land batch-on-partition in PSUM; wemb's
    columns are host-permuted to (l, o, i) order so the DVE mix reads w
    contiguously. Act copies w to SBUF as fp16.
  - Equivariant mix y[b,o,d] = sum_i w[b,(l,o,i)] * x[b,(l,d,i)] runs on DVE
    in fp16 with packed-innermost access patterns (2x perf mode): one
    broadcast multiply per l-block building t[b,(d,o,i)], then out-of-place
    ping-pong fold-adds over i; the last fold writes f32 directly into the
    output layout.
  - All input transposes + inv readout share one PSUM bank per tile.
  - e3nn 1/sqrt(32) norm folded into wemb on host.
"""

import numpy as np
import ml_dtypes

B = 65536
NCORES = 8
BPC = B // NCORES          # 8192 samples per core
NT = BPC // 128            # 64 tiles per core
S_IN = 256
S_OUT = 128
MUL = 32
COND = 64
D1, D2 = 3, 5

_cache = {}


def _build():
    from contextlib import ExitStack
    import concourse.bass as bass
    from concourse import bacc
    import concourse.tile as tile
    import concourse.mybir as mybir

    f32 = mybir.dt.float32
    f16 = mybir.dt.float16
    AF = mybir.ActivationFunctionType

    nc = bacc.Bacc()

    x_d = nc.dram_tensor("x", [BPC, 512], f32, kind="ExternalInput")
    c_d = nc.dram_tensor("cond", [BPC, COND], f32, kind="ExternalInput")
    f1w_d = nc.dram_tensor("f1w", [COND + 1, 512], f16, kind="ExternalInput")
    f2w_d = nc.dram_tensor("f2w", [COND + 1, 512], f16, kind="ExternalInput")
    fcw_d = nc.dram_tensor("fcw", [S_IN, S_IN], f16, kind="ExternalInput")
    invw_d = nc.dram_tensor("invw", [S_IN, S_OUT], f16, kind="ExternalInput")
    wembw_d = nc.dram_tensor("wembw", [S_IN, 2048], f16, kind="ExternalInput")
    ident_d = nc.dram_tensor("ident", [128, 128], f32, kind="ExternalInput")
    out_d = nc.dram_tensor("out", [BPC, 384], f32, kind="ExternalOutput")

    with tile.TileContext(nc) as tc, ExitStack() as ctx:
        wpool = ctx.enter_context(tc.tile_pool(name="weights", bufs=1))
        # --- persistent weights in SBUF ---
        f1w = wpool.tile([COND + 1, 512], f16)
        nc.sync.dma_start(f1w[:], f1w_d[:])
        f2w = wpool.tile([COND + 1, 512], f16)
        nc.sync.dma_start(f2w[:], f2w_d[:])
        # fc1_w [256,256] -> [128, (kchunk 2, m 256)]
        fcw = wpool.tile([128, 512], f16)
        nc.sync.dma_start(fcw[:].rearrange("k (c m) -> k c m", c=2),
                          fcw_d.rearrange("(c k) m -> k c m", c=2))
        invw = wpool.tile([128, 256], f16)
        nc.sync.dma_start(invw[:].rearrange("k (c m) -> k c m", c=2),
                          invw_d.rearrange("(c k) m -> k c m", c=2))
        wemb = wpool.tile([128, 4096], f16)
        nc.sync.dma_start(wemb[:].rearrange("k (c m) -> k c m", c=2),
                          wembw_d.rearrange("(c k) m -> k c m", c=2))
        ident = wpool.tile([128, 128], f32)
        nc.sync.dma_start(ident[:], ident_d[:])

        inp = ctx.enter_context(tc.tile_pool(name="inp", bufs=3))
        act = ctx.enter_context(tc.tile_pool(name="act", bufs=3))
        wsb = ctx.enter_context(tc.tile_pool(name="wsb", bufs=2))
        tpl = ctx.enter_context(tc.tile_pool(name="tpl", bufs=2))
        outp = ctx.enter_context(tc.tile_pool(name="outp", bufs=3))
        ps_in = ctx.enter_context(tc.tile_pool(name="ps_in", bufs=2, space="PSUM"))
        ps_g = ctx.enter_context(tc.tile_pool(name="ps_g", bufs=2, space="PSUM"))
        ps_w = ctx.enter_context(tc.tile_pool(name="ps_w", bufs=1, space="PSUM"))

        for t in range(NT):
            n0 = t * 128
            feat = inp.tile([128, 512], f32, tag="feat")
            nc.sync.dma_start(feat[:], x_d[n0:n0 + 128, :])
            cnd = inp.tile([128, COND], f32, tag="cnd")
            nc.sync.dma_start(cnd[:], c_d[n0:n0 + 128, :])

            # equiv part -> xd[b, (l, d, i)] fp16 (batch-on-partition), Act engine
            xd = act.tile([128, 256], f16, tag="xd")
            for l, D, src0, dst0 in ((0, D1, 256, 0), (1, D2, 256 + MUL * D1, MUL * D1)):
                nc.scalar.activation(
                    xd[:, dst0:dst0 + MUL * D].rearrange("p (d i) -> p d i", i=MUL),
                    feat[:, src0:src0 + MUL * D].rearrange("p (i d) -> p d i", d=D),
                    AF.Copy)

            # input transposes share one PSUM bank, released early.
            T = ps_in.tile([128, 384], f32, tag="T")
            for cblk in range(2):
                nc.tensor.transpose(T[:, 128 * cblk:128 * (cblk + 1)],
                                    feat[:, 128 * cblk:128 * (cblk + 1)], ident[:])
            s0T = act.tile([128, 256], f16, tag="s0T")
            nc.scalar.activation(s0T[:], T[:, 0:256], AF.Copy)
            nc.tensor.transpose(T[:COND, 256:384], cnd[:], ident[:])
            cT = act.tile([COND + 1, 128], f16, tag="cT")
            nc.scalar.activation(cT[:COND, :], T[:COND, 256:384], AF.Copy)
            nc.gpsimd.memset(cT[COND:COND + 1, :], 1.0)

            # --- FiLM1: gb1 = [film1_w; film1_b].T @ [cT; ones] ---
            gb1 = ps_g.tile([128, 512], f32, tag="gb")
            for m in range(4):
                nc.tensor.matmul(gb1[:, 128 * m:128 * (m + 1)],
                                 f1w[:, 128 * m:128 * (m + 1)], cT[:],
                                 start=True, stop=True)
            gbs1 = act.tile([128, 512], f16, tag="gbs1")
            nc.scalar.activation(gbs1[:], gb1[:], AF.Copy)
            s1T = act.tile([128, 256], f16, tag="s1T")
            nc.gpsimd.tensor_mul(s1T[:], s0T[:], gbs1[:, 0:256])
            nc.gpsimd.tensor_add(s1T[:], s1T[:], gbs1[:, 256:512])

            # --- fc1: s2 = fc1_w.T @ s1T ---
            s2 = ps_g.tile([128, 512], f32, tag="gb")
            for m in range(2):
                for c in range(2):
                    nc.tensor.matmul(s2[:, 128 * m:128 * (m + 1)],
                                     fcw[:, 256 * c + 128 * m:256 * c + 128 * (m + 1)],
                                     s1T[:, 128 * c:128 * (c + 1)],
                                     start=(c == 0), stop=(c == 1))
            s2sb = act.tile([128, 256], f16, tag="s2sb")
            nc.scalar.activation(s2sb[:], s2[:, 0:256], AF.Copy)

            # --- FiLM2 ---
            gb2 = ps_g.tile([128, 512], f32, tag="gb")
            for m in range(4):
                nc.tensor.matmul(gb2[:, 128 * m:128 * (m + 1)],
                                 f2w[:, 128 * m:128 * (m + 1)], cT[:],
                                 start=True, stop=True)
            gbs2 = act.tile([128, 512], f16, tag="gbs2")
            nc.scalar.activation(gbs2[:], gb2[:], AF.Copy)
            s3T = act.tile([128, 256], f16, tag="s3T")
            nc.gpsimd.tensor_mul(s3T[:], s2sb[:], gbs2[:, 0:256])
            nc.gpsimd.tensor_add(s3T[:], s3T[:], gbs2[:, 256:512])

            # --- invariant readout + transpose back (PSUM via gb rotation) ---
            To = ps_g.tile([128, 512], f32, tag="gb")
            for c in range(2):
                nc.tensor.matmul(To[:, 0:128], invw[:, 128 * c:128 * (c + 1)],
                                 s3T[:, 128 * c:128 * (c + 1)],
                                 start=(c == 0), stop=(c == 1))
            oT = act.tile([128, 128], f32, tag="oT")
            nc.scalar.activation(oT[:], To[:, 0:128], AF.Copy)
            ostage = outp.tile([128, 384], f32, tag="ostage")
            nc.tensor.transpose(To[:, 128:256], oT[:], ident[:])
            nc.scalar.activation(ostage[:, 0:128], To[:, 128:256], AF.Copy)

            # --- hypernet weights: w = s3 @ wemb, batch-on-partition ---
            w_sb = wsb.tile([128, 2048], f16, tag="w_sb")
            for half in range(2):
                wps = ps_w.tile([128, 1024], f32, tag=f"wps{half}")
                for c in range(2):
                    for s in range(2):
                        nc.tensor.matmul(wps[:, 512 * s:512 * (s + 1)],
                                         s3T[:, 128 * c:128 * (c + 1)],
                                         wemb[:, 2048 * c + 1024 * half + 512 * s:
                                              2048 * c + 1024 * half + 512 * (s + 1)],
                                         start=(c == 0), stop=(c == 1))
                nc.scalar.activation(w_sb[:, 1024 * half:1024 * (half + 1)], wps[:], AF.Copy)

            # --- per-sample equivariant mix on DVE (fp16, 2x perf mode) ---
            # ta[b, d, o, i] = w[b, (l,o,i)] * xd[b, (l,d,i)], then ping-pong
            # out-of-place fold-adds over i. The two l-block chains are
            # interleaved so consecutive DVE instrs are independent; the two
            # smallest folds go to GpSimd.
            st = {}
            for l, D, woff, xoff, yoff in ((0, D1, 0, 0, 128), (1, D2, 1024, MUL * D1, 128 + MUL * D1)):
                ta = tpl.tile([128, D * 1024], f16, tag=f"ta{l}")
                tb = tpl.tile([128, D * 512], f16, tag=f"tb{l}")
                st[l] = (ta, tb, D, woff, xoff, yoff)

            def mul(l):
                ta, tb, D, woff, xoff, yoff = st[l]
                tav = ta[:].rearrange("p (d o i) -> p d o i", o=MUL, i=MUL)
                wv = w_sb[:, woff:woff + 1024].rearrange("p (o i) -> p o i", i=MUL) \
                    .unsqueeze(1).broadcast_to((128, D, MUL, MUL))
                xv = xd[:, xoff:xoff + MUL * D].rearrange("p (d i) -> p d i", i=MUL) \
                    .unsqueeze(2).broadcast_to((128, D, MUL, MUL))
                nc.vector.tensor_mul(tav, wv, xv)

            def fold(l, level):
                ta, tb, D, woff, xoff, yoff = st[l]
                h = MUL >> level          # output i-extent of this fold
                src, dst = (ta, tb) if level % 2 == 1 else (tb, ta)
                sv = src[:, 0:D * 64 * h].rearrange("p (d o i) -> p d o i", o=MUL, i=2 * h)
                if h >= 2:
                    dv = dst[:, 0:D * 32 * h].rearrange("p (d o i) -> p d o i", o=MUL, i=h)
                    nc.vector.tensor_add(dv, sv[:, :, :, 0:h], sv[:, :, :, h:2 * h])
                else:
                    yv = ostage[:, yoff:yoff + MUL * D] \
                        .rearrange("p (o d) -> p d o", d=D).unsqueeze(3)
                    nc.vector.tensor_add(yv, sv[:, :, :, 0:1], sv[:, :, :, 1:2])

            mul(0)
            mul(1)
            for level in range(1, 6):
                fold(0, level)
                fold(1, level)

            nc.sync.dma_start(out_d[n0:n0 + 128, :], ostage[:])

    return nc


def kernel(features, conditioning_tensor, film1_w, film1_b, fc1_w,
           film2_w, film2_b, inv_w, wemb_w):
    from concourse.bass_utils import run_bass_kernel_spmd

    if "nc" not in _cache:
        nc = _build()
        if not nc.is_finalized():
            nc.finalize()
        _cache["nc"] = nc
    nc = _cache["nc"]

    norm = np.float32(1.0 / np.sqrt(MUL))
    feats = np.ascontiguousarray(np.asarray(features, np.float32))
    conds = np.ascontiguousarray(np.asarray(conditioning_tensor, np.float32))

    # wemb columns permuted to (l, o, i) layout; norm folded in.
    wemb_np = np.asarray(wemb_w, np.float32) * norm
    perm = np.empty(2048, dtype=np.int64)
    for l in range(2):
        base = l * MUL * MUL
        for o in range(MUL):
            for i in range(MUL):
                perm[base + o * MUL + i] = base + i * MUL + o
    wemb_perm = wemb_np[:, perm]

    f1wa = np.concatenate([np.asarray(film1_w, np.float32),
                           np.asarray(film1_b, np.float32)[None, :]], axis=0)
    f2wa = np.concatenate([np.asarray(film2_w, np.float32),
                           np.asarray(film2_b, np.float32)[None, :]], axis=0)

    shared = {
        "f1w": f1wa.astype(np.float16),
        "f2w": f2wa.astype(np.float16),
        "fcw": np.asarray(fc1_w, np.float32).astype(np.float16),
        "invw": np.asarray(inv_w, np.float32).astype(np.float16),
        "wembw": wemb_perm.astype(np.float16),
        "ident": np.eye(128, dtype=np.float32),
    }
    in_maps = []
    for i in range(NCORES):
        m = dict(shared)
        m["x"] = feats[i * BPC:(i + 1) * BPC]
        m["cond"] = conds[i * BPC:(i + 1) * BPC]
        in_maps.append(m)

    import os
    trace = bool(int(os.environ.get("KERNEL_TRACE", "0")))
    res = run_bass_kernel_spmd(nc, in_maps, core_ids=list(range(NCORES)), trace=trace)
    _cache["last"] = res
    return np.concatenate([r["out"] for r in res.results], axis=0)
